# revision 9
# baseline (speedup 1.0000x reference)
"""Trainium2 Bass kernel for nn_Local_align: per-sample dynamic 3x3 conv.

  img = l2norm(vision, axis=C)                              [B,C,H,W]
  tf  = l2norm(text, axis=-1) @ Wt.T + bt                   [B,Nc,out_dim]
  w   = softmax(tf[..., :-1] grouped per (C, 3x3)), b = tf[..., -1]
  out[b] = conv2d_same(img[b], w[b]) + b                    [B,Nc,H,W]

Sharding: data-parallel over batch B=8, one image per NeuronCore.
"""

import numpy as np

B = 8
C, H, W = 256, 128, 128
NC, KD = 150, 768
KK = 9  # 3x3 taps
OD = C * KK + 1  # 2305
HW = H * W  # 16384
PW = W + 2  # padded row width 130
PH = H + 2
WS = 512  # pixel window = 4 rows
NWIN = HW // WS  # 32
ROWS_PER_WIN = WS // W  # 4
NC_CHUNKS = [(0, 128), (128, 22)]
KCH = KD // 128  # 6
CCH = C // 128  # 2
# out_dim windows for the text matmul
TF_WINS = [(0, 512), (512, 512), (1024, 512), (1536, 512), (2048, 257)]


def _build_program(reps=1):
    import concourse.bacc as bacc
    import concourse.tile as tile
    from concourse import mybir
    from concourse.masks import make_identity

    f32 = mybir.dt.float32
    f32r = mybir.dt.float32r
    f16 = mybir.dt.float16
    MUL = mybir.AluOpType.mult
    AX = mybir.ActivationFunctionType

    nc = bacc.Bacc("TRN2", target_bir_lowering=False, debug=False)

    vis = nc.dram_tensor("vision", [C, HW], f32, kind="ExternalInput").ap()
    txt = nc.dram_tensor("text", [NC, KD], f32, kind="ExternalInput").ap()
    wtT = nc.dram_tensor("wtT", [KD, OD], f16, kind="ExternalInput").ap()
    btd = nc.dram_tensor("bt", [OD], f16, kind="ExternalInput").ap()
    out = nc.dram_tensor("out", [NC, HW], f32, kind="ExternalOutput").ap()

    with tile.TileContext(nc) as tc:
        with (
            tc.tile_pool(name="singles", bufs=1) as singles,
            tc.tile_pool(name="persist", bufs=1) as persist,
            tc.tile_pool(name="pa", bufs=1) as pa,
            tc.tile_pool(name="rawp", bufs=4) as rawp,
            tc.tile_pool(name="sqp", bufs=2) as sqp,
            tc.tile_pool(name="nrm", bufs=2) as nrm,
            tc.tile_pool(name="outp", bufs=3) as outp,
            tc.tile_pool(name="pst", bufs=2, space="PSUM") as pstp,
            tc.tile_pool(name="psw", bufs=2, space="PSUM") as pswp,
            tc.tile_pool(name="pss", bufs=2, space="PSUM") as pssp,
            tc.tile_pool(name="pso", bufs=2, space="PSUM") as psop,
        ):
            # ---------------- constants ----------------
            ident32 = singles.tile([128, 128], f32)
            make_identity(nc, ident32)
            ident16 = singles.tile([128, 128], f16)
            make_identity(nc, ident16)
            ones32f = singles.tile([128, 128], f32)
            nc.gpsimd.memset(ones32f, 1.0)
            ones32 = singles.tile([128, 128], f32r)
            nc.scalar.copy(ones32, ones32f)
            ones16 = singles.tile([1, 128], f16)
            nc.gpsimd.memset(ones16, 1.0)
            bt16 = singles.tile([1, OD], f16)
            nc.sync.dma_start(out=bt16, in_=btd.unsqueeze(0))

            # ---------------- kernel body ----------------
            # (hw-looped `reps` times for benchmarking; reps=1 is the real kernel)
            args = (
                nc, tc, mybir, persist, pa, rawp, sqp, nrm, outp,
                pstp, pswp, pssp, psop,
                vis, txt, wtT, btd, out,
                ident32, ident16, ones32, ones16, bt16,
            )
            if reps == 1:
                _emit_iteration(*args)
            else:
                with tc.For_i(0, reps, 1):
                    _emit_iteration(*args)

    nc.compile()
    return nc


def _emit_iteration(nc, tc, mybir, persist, pa, rawp, sqp, nrm, outp,
                    pstp, pswp, pssp, psop,
                    vis, txt, wtT, btd, out,
                    ident32, ident16, ones32, ones16, bt16):
    f32 = mybir.dt.float32
    f32r = mybir.dt.float32r
    f16 = mybir.dt.float16
    MUL = mybir.AluOpType.mult
    AX = mybir.ActivationFunctionType

    if True:
        if True:
            # ---------------- persistent tiles ----------------
            pads = [persist.tile([128, PH, PW], f16, name=f"pad{i}") for i in range(CCH)]
            wTs = [persist.tile([128, KK, NC], f16, name=f"wT{i}") for i in range(CCH)]
            t_hatT = persist.tile([128, KCH, NC], f16)
            tfs = [persist.tile([128, OD], f32, name=f"tf{i}") for i in range(len(NC_CHUNKS))]
            wtT16 = persist.tile([128, KCH, OD], f16)

            # zero the halo borders of the padded images
            for p in pads:
                nc.gpsimd.memset(p[:, 0, :], 0.0)
                nc.gpsimd.memset(p[:, PH - 1, :], 0.0)
                nc.gpsimd.memset(p[:, :, 0], 0.0)
                nc.gpsimd.memset(p[:, :, PW - 1], 0.0)

            # ---------------- vision: load + channel-l2 normalize ----------------
            # window = 4 image rows (512 px). sumsq over all 256 channels via an
            # all-ones [128,128] stationary matmul -> result replicated on all
            # 128 partitions (reduce + broadcast in one shot).
            for w in range(NWIN):
                r0 = w * ROWS_PER_WIN
                raws = []
                for cc in range(CCH):
                    raw = rawp.tile([128, WS], f32, name=f"raw{cc}", tag=f"raw{cc}")
                    nc.sync.dma_start(out=raw, in_=vis[cc * 128:(cc + 1) * 128, w * WS:(w + 1) * WS])
                    raws.append(raw)
                ps = pssp.tile([128, WS], f32, tag="pss")
                for cc in range(CCH):
                    sq = sqp.tile([128, WS], f32r, name=f"sq{cc}", tag=f"sq{cc}")
                    nc.scalar.square(sq, raws[cc])
                    nc.tensor.matmul(
                        ps,
                        ones32,
                        sq,
                        start=(cc == 0),
                        stop=(cc == CCH - 1),
                    )
                sqv = nrm.tile([128, WS], f32, tag="sqv")
                nc.scalar.sqrt(sqv, ps)
                inv = nrm.tile([128, WS], f32, tag="inv")
                nc.vector.reciprocal(inv, sqv)
                inv_v = inv.rearrange("p (r x) -> p r x", x=W)
                for cc in range(CCH):
                    nc.vector.tensor_tensor(
                        out=pads[cc][:, 1 + r0:1 + r0 + ROWS_PER_WIN, 1:1 + W],
                        in0=raws[cc].rearrange("p (r x) -> p r x", x=W),
                        in1=inv_v,
                        op=MUL,
                    )

            # ---------------- text path ----------------
            t_sbs = []
            for i, (n0, cnt) in enumerate(NC_CHUNKS):
                t_sb = pa.tile([128, KD], f32, name=f"t_sb{i}")
                nc.sync.dma_start(out=t_sb[:cnt], in_=txt[n0:n0 + cnt])
                t_sbs.append(t_sb)
            nc.sync.dma_start(out=wtT16, in_=wtT.rearrange("(k p) o -> p k o", p=128))

            tsq = pa.tile([128, KD], f32)
            stat = pa.tile([128, 4], f32)
            for i, (n0, cnt) in enumerate(NC_CHUNKS):
                t_sb = t_sbs[i]
                nc.scalar.activation(
                    out=tsq[:cnt], in_=t_sb[:cnt], func=AX.Square,
                    accum_out=stat[:cnt, 0:1],
                )
                nc.scalar.sqrt(stat[:cnt, 1:2], stat[:cnt, 0:1])
                nc.vector.reciprocal(stat[:cnt, 2:3], stat[:cnt, 1:2])
                nc.vector.tensor_scalar_mul(t_sb[:cnt], t_sb[:cnt], stat[:cnt, 2:3])
                # transpose normalized text -> [KD, nc] (fp16)
                for k in range(KCH):
                    pst = pstp.tile([128, 128], f32, name="pst", tag="pst")
                    nc.tensor.transpose(
                        pst[:, :cnt], t_sb[:cnt, k * 128:(k + 1) * 128], ident32[:cnt, :cnt]
                    )
                    nc.scalar.copy(t_hatT[:, k, n0:n0 + cnt], pst[:, :cnt])

            # tf = t_hat @ Wt.T + bt   (fp16 inputs, fp32 accum)
            for i, (n0, cnt) in enumerate(NC_CHUNKS):
                for (o0, ws) in TF_WINS:
                    psw = pswp.tile([128, 512], f32, tag="psw")
                    for k in range(KCH):
                        nc.tensor.matmul(
                            psw[:cnt, :ws],
                            t_hatT[:, k, n0:n0 + cnt],
                            wtT16[:, k, o0:o0 + ws],
                            start=(k == 0),
                            stop=False,
                        )
                    nc.tensor.matmul(
                        psw[:cnt, :ws],
                        ones16[:1, :cnt],
                        bt16[:1, o0:o0 + ws],
                        start=False,
                        stop=True,
                    )
                    nc.scalar.copy(tfs[i][:cnt, o0:o0 + ws], psw[:cnt, :ws])

            # softmax over the 9 taps per (nc, c); conv bias stays at tf[:, 2304]
            for i, (n0, cnt) in enumerate(NC_CHUNKS):
                tfw = tfs[i][:cnt, 0:C * KK]
                nc.scalar.activation(out=tfw, in_=tfw, func=AX.Exp)
                tfv = tfw.rearrange("p (c t) -> p c t", t=KK)
                ssum = pa.tile([128, C], f32, tag="ssum")
                nc.vector.reduce_sum(out=ssum[:cnt], in_=tfv, axis=mybir.AxisListType.X)
                rsum = pa.tile([128, C], f32, tag="rsum")
                nc.vector.reciprocal(rsum[:cnt], ssum[:cnt])
                w16 = pa.tile([128, C * KK], f16, name=f"w16_{i}", tag=f"w16_{i}")
                nc.vector.tensor_tensor(
                    out=w16[:cnt].rearrange("p (c t) -> p c t", t=KK),
                    in0=tfv,
                    in1=rsum[:cnt].unsqueeze(2).broadcast_to([cnt, C, KK]),
                    op=MUL,
                )
                # transpose weights to [c, nc] per tap (fp16)
                w16v = w16.rearrange("p (c t) -> p c t", t=KK)
                for cc in range(CCH):
                    for tap in range(KK):
                        pst = pstp.tile([128, 128], f16, name="pst", tag="pst")
                        nc.tensor.transpose(
                            pst[:, :cnt],
                            w16v[:cnt, cc * 128:(cc + 1) * 128, tap],
                            ident16[:cnt, :cnt],
                        )
                        nc.scalar.copy(wTs[cc][:, tap, n0:n0 + cnt], pst[:, :cnt])

            # ---------------- conv: 18 accumulating matmuls per psum tile ----------------
            for w in range(NWIN):
                r0 = w * ROWS_PER_WIN
                for i, (n0, cnt) in enumerate(NC_CHUNKS):
                    ps = psop.tile([128, WS], f32, tag="pso")
                    mi = 0
                    for cc in range(CCH):
                        for ty in range(3):
                            for tx in range(3):
                                nc.tensor.matmul(
                                    ps[:cnt],
                                    wTs[cc][:, ty * 3 + tx, n0:n0 + cnt],
                                    pads[cc][:, r0 + ty:r0 + ty + ROWS_PER_WIN, tx:tx + W],
                                    start=(mi == 0),
                                    stop=(mi == CCH * KK - 1),
                                )
                                mi += 1
                    osb = outp.tile([128, WS], f32, tag="osb")
                    nc.scalar.add(osb[:cnt], ps[:cnt], add=tfs[i][:cnt, C * KK:C * KK + 1])
                    nc.sync.dma_start(
                        out=out[n0:n0 + cnt, w * WS:(w + 1) * WS], in_=osb[:cnt]
                    )


_NC_CACHE = {}


def _get_program(reps=1):
    if reps not in _NC_CACHE:
        _NC_CACHE[reps] = _build_program(reps)
    return _NC_CACHE[reps]


def _make_in_maps(vision, text, Wt, bt):
    wtT16 = np.ascontiguousarray(Wt.astype(np.float32).T).astype(np.float16)
    bt16 = bt.astype(np.float16)
    in_maps = []
    for b in range(B):
        in_maps.append({
            "vision": np.ascontiguousarray(vision[b].reshape(C, HW)),
            "text": np.ascontiguousarray(text[b, :, 0, :]),
            "wtT": wtT16,
            "bt": bt16,
        })
    return in_maps


def _run(vision, text, Wt, bt, trace=False):
    from concourse.bass_utils import run_bass_kernel_spmd

    nc = _get_program()
    in_maps = _make_in_maps(vision, text, Wt, bt)
    res = run_bass_kernel_spmd(nc, in_maps, list(range(B)), trace=trace)
    outs = np.stack([np.asarray(res.results[b]["out"]).reshape(NC, H, W) for b in range(B)])
    return outs, res


def kernel(vision, text, Wt, bt):
    outs, _ = _run(vision, text, Wt, bt, trace=False)
    return outs


# revision 18
# speedup vs baseline: 5.2078x; 5.2078x over previous
"""Trainium2 Bass kernel for nn_Local_align: per-sample dynamic 3x3 conv.

  img = l2norm(vision, axis=C)                              [B,C,H,W]
  tf  = l2norm(text, axis=-1) @ Wt.T + bt                   [B,Nc,out_dim]
  w   = softmax(tf[..., :-1] grouped per (C, 3x3)), b = tf[..., -1]
  out[b] = conv2d_same(img[b], w[b]) + b                    [B,Nc,H,W]

Sharding: data-parallel over batch B=8, one image per NeuronCore.
"""

import numpy as np

B = 8
C, H, W = 256, 128, 128
NC, KD = 150, 768
KK = 9  # 3x3 taps
OD = C * KK + 1  # 2305
HW = H * W  # 16384
PW = W + 2  # padded row width 130
PH = H + 2
WS = 512  # pixel window = 4 rows
NWIN = HW // WS  # 32
ROWS_PER_WIN = WS // W  # 4
NC_CHUNKS = [(0, 128), (128, 22)]
KCH = KD // 128  # 6
CCH = C // 128  # 2
# out_dim windows for the text matmul
TF_WINS = [(0, 512), (512, 512), (1024, 512), (1536, 512), (2048, 257)]


def _build_program(reps=1, ablate=frozenset()):
    import concourse.bacc as bacc
    import concourse.tile as tile
    from concourse import mybir
    from concourse.masks import make_identity

    f32 = mybir.dt.float32
    f32r = mybir.dt.float32r
    f16 = mybir.dt.float16
    MUL = mybir.AluOpType.mult
    AX = mybir.ActivationFunctionType

    nc = bacc.Bacc("TRN2", target_bir_lowering=False, debug=False)

    vis = nc.dram_tensor("vision", [C, HW], f32, kind="ExternalInput").ap()
    txt = nc.dram_tensor("text", [NC, KD], f32, kind="ExternalInput").ap()
    wtT = nc.dram_tensor("wtT", [KD, OD], f16, kind="ExternalInput").ap()
    btd = nc.dram_tensor("bt", [OD], f16, kind="ExternalInput").ap()
    out = nc.dram_tensor("out", [NC, HW], f32, kind="ExternalOutput").ap()

    with tile.TileContext(nc) as tc:
        with (
            tc.tile_pool(name="singles", bufs=1) as singles,
            tc.tile_pool(name="persist", bufs=1) as persist,
            tc.tile_pool(name="pa", bufs=1) as pa,
            tc.tile_pool(name="rawp", bufs=4) as rawp,
            tc.tile_pool(name="sqp", bufs=2) as sqp,
            tc.tile_pool(name="nrm", bufs=2) as nrm,
            tc.tile_pool(name="outp", bufs=3) as outp,
            tc.tile_pool(name="pst", bufs=2, space="PSUM") as pstp,
            tc.tile_pool(name="psw", bufs=2, space="PSUM") as pswp,
            tc.tile_pool(name="pss", bufs=2, space="PSUM") as pssp,
            tc.tile_pool(name="pso", bufs=2, space="PSUM") as psop,
        ):
            # ---------------- constants ----------------
            ident32 = singles.tile([128, 128], f32)
            make_identity(nc, ident32)
            ident16 = singles.tile([128, 128], f16)
            make_identity(nc, ident16)
            ones32f = singles.tile([128, 128], f32)
            nc.gpsimd.memset(ones32f, 1.0)
            ones32 = singles.tile([128, 128], f32r)
            nc.scalar.copy(ones32, ones32f)
            ones16 = singles.tile([1, 128], f16)
            nc.gpsimd.memset(ones16, 1.0)
            bt16 = singles.tile([1, OD], f16)
            nc.sync.dma_start(out=bt16, in_=btd.unsqueeze(0))

            # ---------------- kernel body ----------------
            # (hw-looped `reps` times for benchmarking; reps=1 is the real kernel)
            args = (
                nc, tc, mybir, persist, pa, rawp, sqp, nrm, outp,
                pstp, pswp, pssp, psop,
                vis, txt, wtT, btd, out,
                ident32, ident16, ones32, ones16, bt16, ablate,
            )
            if reps == 1:
                _emit_iteration(*args)
            else:
                with tc.For_i(0, reps, 1):
                    _emit_iteration(*args)

    nc.compile()
    return nc


def _emit_iteration(nc, tc, mybir, persist, pa, rawp, sqp, nrm, outp,
                    pstp, pswp, pssp, psop,
                    vis, txt, wtT, btd, out,
                    ident32, ident16, ones32, ones16, bt16, ablate=frozenset()):
    f32 = mybir.dt.float32
    f32r = mybir.dt.float32r
    f16 = mybir.dt.float16
    MUL = mybir.AluOpType.mult
    AX = mybir.ActivationFunctionType

    if True:
        if True:
            # ---------------- persistent tiles ----------------
            pads = [persist.tile([128, PH, PW], f16, name=f"pad{i}") for i in range(CCH)]
            wTs = [persist.tile([128, KK, NC], f16, name=f"wT{i}") for i in range(CCH)]
            t_hatT = persist.tile([128, KCH, NC], f16)
            tfs = [persist.tile([128, OD], f32, name=f"tf{i}") for i in range(len(NC_CHUNKS))]
            wtT16 = persist.tile([128, KCH, OD], f16)

            # zero the halo borders of the padded images
            for p in pads:
                nc.gpsimd.memset(p[:, 0, :], 0.0)
                nc.gpsimd.memset(p[:, PH - 1, :], 0.0)
                nc.gpsimd.memset(p[:, :, 0], 0.0)
                nc.gpsimd.memset(p[:, :, PW - 1], 0.0)

            # ---------------- vision: load + channel-l2 normalize ----------------
            # window = 4 image rows (512 px). sumsq over all 256 channels via an
            # all-ones [128,128] stationary matmul -> result replicated on all
            # 128 partitions (reduce + broadcast in one shot).
            for w in range(NWIN):
                if "norm" in ablate and "visdma" in ablate:
                    break
                r0 = w * ROWS_PER_WIN
                raws = []
                for cc in range(CCH):
                    raw = rawp.tile([128, WS], f32, name=f"raw{cc}", tag=f"raw{cc}")
                    if "visdma" not in ablate:
                        nc.sync.dma_start(out=raw, in_=vis[cc * 128:(cc + 1) * 128, w * WS:(w + 1) * WS])
                    raws.append(raw)
                if "norm" in ablate:
                    continue
                ps = pssp.tile([128, WS], f32, tag="pss")
                for cc in range(CCH):
                    sq = sqp.tile([128, WS], f32r, name=f"sq{cc}", tag=f"sq{cc}")
                    nc.scalar.square(sq, raws[cc])
                    nc.tensor.matmul(
                        ps,
                        ones32,
                        sq,
                        start=(cc == 0),
                        stop=(cc == CCH - 1),
                    )
                sqv = nrm.tile([128, WS], f32, tag="sqv")
                nc.scalar.sqrt(sqv, ps)
                inv = nrm.tile([128, WS], f32, tag="inv")
                nc.vector.reciprocal(inv, sqv)
                inv_v = inv.rearrange("p (r x) -> p r x", x=W)
                for cc in range(CCH):
                    nc.vector.tensor_tensor(
                        out=pads[cc][:, 1 + r0:1 + r0 + ROWS_PER_WIN, 1:1 + W],
                        in0=raws[cc].rearrange("p (r x) -> p r x", x=W),
                        in1=inv_v,
                        op=MUL,
                    )

            # ---------------- text path ----------------
            if "phasea" in ablate:
                return
            t_sbs = []
            for i, (n0, cnt) in enumerate(NC_CHUNKS):
                t_sb = pa.tile([128, KD], f32, name=f"t_sb{i}")
                nc.sync.dma_start(out=t_sb[:cnt], in_=txt[n0:n0 + cnt])
                t_sbs.append(t_sb)
            nc.sync.dma_start(out=wtT16, in_=wtT.rearrange("(k p) o -> p k o", p=128))

            tsq = pa.tile([128, KD], f32)
            stat = pa.tile([128, 4], f32)
            for i, (n0, cnt) in enumerate(NC_CHUNKS):
                t_sb = t_sbs[i]
                nc.scalar.activation(
                    out=tsq[:cnt], in_=t_sb[:cnt], func=AX.Square,
                    accum_out=stat[:cnt, 0:1],
                )
                nc.scalar.sqrt(stat[:cnt, 1:2], stat[:cnt, 0:1])
                nc.vector.reciprocal(stat[:cnt, 2:3], stat[:cnt, 1:2])
                nc.vector.tensor_scalar_mul(t_sb[:cnt], t_sb[:cnt], stat[:cnt, 2:3])
                # transpose normalized text -> [KD, nc] (fp16)
                for k in range(KCH):
                    pst = pstp.tile([128, 128], f32, name="pst", tag="pst")
                    nc.tensor.transpose(
                        pst[:, :cnt], t_sb[:cnt, k * 128:(k + 1) * 128], ident32[:cnt, :cnt]
                    )
                    nc.scalar.copy(t_hatT[:, k, n0:n0 + cnt], pst[:, :cnt])

            # tf = t_hat @ Wt.T + bt   (fp16 inputs, fp32 accum)
            for i, (n0, cnt) in enumerate(NC_CHUNKS):
                for (o0, ws) in TF_WINS:
                    psw = pswp.tile([128, 512], f32, tag="psw")
                    for k in range(KCH):
                        nc.tensor.matmul(
                            psw[:cnt, :ws],
                            t_hatT[:, k, n0:n0 + cnt],
                            wtT16[:, k, o0:o0 + ws],
                            start=(k == 0),
                            stop=False,
                        )
                    nc.tensor.matmul(
                        psw[:cnt, :ws],
                        ones16[:1, :cnt],
                        bt16[:1, o0:o0 + ws],
                        start=False,
                        stop=True,
                    )
                    nc.scalar.copy(tfs[i][:cnt, o0:o0 + ws], psw[:cnt, :ws])

            # softmax over the 9 taps per (nc, c); conv bias stays at tf[:, 2304]
            for i, (n0, cnt) in enumerate(NC_CHUNKS):
                tfw = tfs[i][:cnt, 0:C * KK]
                nc.scalar.activation(out=tfw, in_=tfw, func=AX.Exp)
                tfv = tfw.rearrange("p (c t) -> p c t", t=KK)
                ssum = pa.tile([128, C], f32, tag="ssum")
                nc.vector.reduce_sum(out=ssum[:cnt], in_=tfv, axis=mybir.AxisListType.X)
                rsum = pa.tile([128, C], f32, tag="rsum")
                nc.vector.reciprocal(rsum[:cnt], ssum[:cnt])
                w16 = pa.tile([128, C * KK], f16, name=f"w16_{i}", tag=f"w16_{i}")
                nc.vector.tensor_tensor(
                    out=w16[:cnt].rearrange("p (c t) -> p c t", t=KK),
                    in0=tfv,
                    in1=rsum[:cnt].unsqueeze(2).broadcast_to([cnt, C, KK]),
                    op=MUL,
                )
                # transpose weights to [c, nc] per tap (fp16)
                w16v = w16.rearrange("p (c t) -> p c t", t=KK)
                for cc in range(CCH):
                    for tap in range(KK):
                        pst = pstp.tile([128, 128], f16, name="pst", tag="pst")
                        nc.tensor.transpose(
                            pst[:, :cnt],
                            w16v[:cnt, cc * 128:(cc + 1) * 128, tap],
                            ident16[:cnt, :cnt],
                        )
                        nc.scalar.copy(wTs[cc][:, tap, n0:n0 + cnt], pst[:, :cnt])

            # ---------------- conv: 18 accumulating matmuls per psum tile ----------------
            if "conv" in ablate:
                return
            for w in range(NWIN):
                r0 = w * ROWS_PER_WIN
                for i, (n0, cnt) in enumerate(NC_CHUNKS):
                    ps = psop.tile([128, WS], f32, tag="pso")
                    mi = 0
                    for cc in range(CCH):
                        for ty in range(3):
                            for tx in range(3):
                                nc.tensor.matmul(
                                    ps[:cnt],
                                    wTs[cc][:, ty * 3 + tx, n0:n0 + cnt],
                                    pads[cc][:, r0 + ty:r0 + ty + ROWS_PER_WIN, tx:tx + W],
                                    start=(mi == 0),
                                    stop=(mi == CCH * KK - 1),
                                )
                                mi += 1
                    osb = outp.tile([128, WS], f32, tag="osb")
                    nc.scalar.add(osb[:cnt], ps[:cnt], add=tfs[i][:cnt, C * KK:C * KK + 1])
                    if "outdma" not in ablate:
                        nc.sync.dma_start(
                            out=out[n0:n0 + cnt, w * WS:(w + 1) * WS], in_=osb[:cnt]
                        )


_NC_CACHE = {}


def _get_program(reps=1, ablate=frozenset()):
    ablate = frozenset(ablate)
    key = (reps, ablate)
    if key not in _NC_CACHE:
        _NC_CACHE[key] = _build_program(reps, ablate)
    return _NC_CACHE[key]


def _make_in_maps(vision, text, Wt, bt):
    wtT16 = np.ascontiguousarray(Wt.astype(np.float32).T).astype(np.float16)
    bt16 = bt.astype(np.float16)
    in_maps = []
    for b in range(B):
        in_maps.append({
            "vision": np.ascontiguousarray(vision[b].reshape(C, HW)),
            "text": np.ascontiguousarray(text[b, :, 0, :]),
            "wtT": wtT16,
            "bt": bt16,
        })
    return in_maps


def _run(vision, text, Wt, bt, trace=False):
    from concourse.bass_utils import run_bass_kernel_spmd

    nc = _get_program()
    in_maps = _make_in_maps(vision, text, Wt, bt)
    res = run_bass_kernel_spmd(nc, in_maps, list(range(B)), trace=trace)
    outs = np.stack([np.asarray(res.results[b]["out"]).reshape(NC, H, W) for b in range(B)])
    return outs, res


def kernel(vision, text, Wt, bt):
    outs, _ = _run(vision, text, Wt, bt, trace=False)
    return outs
